# revision 38
# baseline (speedup 1.0000x reference)
"""Multi-head attention forward (B=2, S=2048, H=2048, 16 heads) on 8 TRN2 NeuronCores.

Sharding: tensor-parallel over heads — 2 heads per core. Each core computes
Q/K/V projections for its 2 heads (full batch), attention, and a partial
output projection (its heads' columns of Wo); the host sums the 8 partial
outputs and adds the bias terms.

Device compute is bf16 with fp32 PSUM accumulation. Host pre-transposes
the activation matrix (X.T) and weight slices so the device never has to
transpose fp32 data (fp32 DMA transpose is unsupported).

Layout notes (matmul computes lhsT.T @ rhs, contracting the partition dim):
  - Q.T, K.T are computed as [head_dim, tokens] (d on partitions):
        lhsT = Wq.T tile [hid, d], rhs = X.T tile [hid, tokens]
  - V is computed natural [tokens, d]: lhsT = X.T tile, rhs = Wv.T tile
  - scores transposed S.T[k_tok, q] = (K.T tile).T @ Q.T  (contract d=128)
  - P.T = exp(SCALE * S.T + mask) via one scalar-engine activation
    (mask is per-key = per-partition, so it rides the activation bias)
  - ctx.T[d, q] = V_tile.T @ P.T (contract k_tok), accumulated over k tiles
  - softmax denominators via ones-vector matmul: [1,q] += ones.T @ P.T
  - out_partial[t, o] = (ctx.T tile).T @ Wo.T tile (contract local head dims)

bv/bo are folded on the host: rows of normalized P sum to 1, so
ctx = P@(V + bv) = P@V + bv, giving out += bv @ Wo.T + bo after the
cross-core reduction.
"""

import os

import numpy as np
import ml_dtypes

P = 128
HIDDEN = 2048
NUM_HEADS = 16
HEAD_DIM = 128
B, S = 2, 2048
T = B * S                     # 4096 tokens
N_CORES = 8
H_LOC = NUM_HEADS // N_CORES  # 2 heads per core
DLOC = H_LOC * HEAD_DIM       # 256
KO = HIDDEN // P              # 16 contraction tiles for the projections
CH = 8                        # token chunks for the projection phase
CHW = T // CH                 # 512 tokens per chunk
NKT = S // P                  # 16 key tiles per batch
NQ = S // 512                 # 4 query tiles (512 wide) per batch
SCALE = float(1.0 / np.sqrt(HEAD_DIM).astype(np.float32))

BF16NP = ml_dtypes.bfloat16

_CACHE = {}


def _split_multi_waits(nc):
    """Split instructions carrying >1 semaphore wait.

    This walrus build rejects any instruction with more than one sync wait
    ("Too many sync wait commands"), but Tile's wait assignment freely
    attaches several. Hoist all but the last wait onto same-engine NOPs
    inserted immediately before the instruction — each engine sequencer
    executes its queue in order, so blocking on a preceding NOP is
    equivalent to blocking on the instruction itself.
    """
    import bass_rust
    import concourse.mybir as mybir

    cnt = 0
    for f in nc.m.functions:
        for bb in f.blocks:
            out = []
            for inst in bb.instructions:
                si = inst.sync_info
                waits = list(si.on_wait) if si and si.on_wait else []
                if len(waits) > 1:
                    for w in waits[:-1]:
                        nop = mybir.InstNoOp(name=f"wsplit_{cnt}", ins=[], outs=[])
                        cnt += 1
                        nop.engine = inst.engine
                        nop.sync_info = bass_rust.SyncInfo(on_wait=[w], on_update=[])
                        out.append(nop)
                    inst.sync_info = bass_rust.SyncInfo(
                        on_wait=[waits[-1]], on_update=list(si.on_update or [])
                    )
                out.append(inst)
            bb.instructions[:] = out
    return cnt


def _build_nc(loop_k=None, **opts):
    """Build the kernel module.

    loop_k: if set, wrap the whole compute body in a For_i running it loop_k
    times — used only for benchmarking (slope timing); the graded kernel
    uses loop_k=None (straight-line body).
    opts: benchmark-only ablation switches (default: all off).
    """
    import concourse.bass as bass
    import concourse.mybir as mybir
    import concourse.tile as tile

    no_sums = opts.get("no_sums", False)
    no_phase3 = opts.get("no_phase3", False)
    no_out_dma = opts.get("no_out_dma", False)
    no_attn = opts.get("no_attn", False)
    xch_bufs = opts.get("xch_bufs", 3)
    pt_bufs = opts.get("pt_bufs", 4)
    norm2 = opts.get("norm2", False)        # deferred norm (early psum drain)
    interleave = opts.get("interleave", False)  # phase 2/3 interleaved per batch
    fuse = opts.get("fuse", False)          # phase 2/3 fused at qi granularity
    vcopy_act = opts.get("vcopy_act", False)  # V psum drain on scalar engine
    norm_gp = opts.get("norm_gp", False)    # norm bounce DMAs on ACT queues
    xt_gp = opts.get("xt_gp", False)        # xt streaming loads on ACT queues
    norm3 = opts.get("norm3", False)        # reciprocal broadcast via PE matmul
    sched2 = opts.get("sched2", False)      # global proj/attn/outproj interleave
    stagger = opts.get("stagger", 0)        # ctx MM issued N steps behind S.T
    act_lite = opts.get("act_lite", False)  # keep ACT for exps only
    early_x = opts.get("early_x", False)    # first x chunk loads before consts
    sums_defer = opts.get("sums_defer", False)  # sums MMs after the kt loop
    sums_tree = opts.get("sums_tree", None)  # "gpsimd"|"vector": adder tree
    sums_acc = opts.get("sums_acc", None)   # "gpsimd"|"vector": inline accum
    ps_bufs = opts.get("ps_bufs", 8)
    out_bf16 = opts.get("out_bf16", False)  # bf16 partial output
    obp_bufs = opts.get("obp_bufs", 3)
    split_in = opts.get("split_in", False)  # split startup DMAs for fast ramp

    fp32 = mybir.dt.float32
    bf16 = mybir.dt.bfloat16

    nc = bass.Bass()

    xt_d = nc.dram_tensor("xt", [HIDDEN, T], bf16, kind="ExternalInput")
    wqt_d = nc.dram_tensor("wqt", [HIDDEN, DLOC], bf16, kind="ExternalInput")
    wkt_d = nc.dram_tensor("wkt", [HIDDEN, DLOC], bf16, kind="ExternalInput")
    wvt_d = nc.dram_tensor("wvt", [HIDDEN, DLOC], bf16, kind="ExternalInput")
    wot_d = nc.dram_tensor("wot", [DLOC, HIDDEN], bf16, kind="ExternalInput")
    bq_d = nc.dram_tensor("bq", [DLOC], fp32, kind="ExternalInput")
    bk_d = nc.dram_tensor("bk", [DLOC], fp32, kind="ExternalInput")
    mask_d = nc.dram_tensor("mask", [B, S], fp32, kind="ExternalInput")
    out_dt = bf16 if out_bf16 else fp32
    out_d = nc.dram_tensor("out", [T, HIDDEN], out_dt, kind="ExternalOutput")

    xt_v = xt_d[:].rearrange("(ko p) t -> p ko t", p=P)
    wqt_v = wqt_d[:].rearrange("(ko p) d -> p ko d", p=P)
    wkt_v = wkt_d[:].rearrange("(ko p) d -> p ko d", p=P)
    wvt_v = wvt_d[:].rearrange("(ko p) d -> p ko d", p=P)
    wot_v = wot_d[:].rearrange("(h p) o -> p h o", p=P)
    bq_v = bq_d[:].rearrange("(h p) -> p h", p=P)
    bk_v = bk_d[:].rearrange("(h p) -> p h", p=P)
    mask_v = mask_d[:].rearrange("b (ko p) -> p b ko", p=P)

    with tile.TileContext(nc) as tc:
        with (
            tc.tile_pool(name="const", bufs=1) as const,
            tc.tile_pool(name="big", bufs=1) as big,
            tc.tile_pool(name="xch", bufs=xch_bufs) as xch,
            tc.tile_pool(name="ptp", bufs=pt_bufs) as ptp,
            tc.tile_pool(name="nrm", bufs=opts.get("nrm_bufs", 2)) as nrm,
            tc.tile_pool(name="ob", bufs=obp_bufs) as obp,
            tc.tile_pool(name="ps", bufs=ps_bufs, space="PSUM") as psp,
            tc.tile_pool(name="dscr", bufs=4, space="DRAM") as dscr,
        ):
            Ident = mybir.ActivationFunctionType.Identity
            Exp = mybir.ActivationFunctionType.Exp

            def ps_tile():
                return psp.tile([P, 512], fp32, tag="ps", name="ps")

            # ---- resident constants -------------------------------------
            wq_sb = const.tile([P, KO, DLOC], bf16)
            wk_sb = const.tile([P, KO, DLOC], bf16)
            wv_sb = const.tile([P, KO, DLOC], bf16)
            wo_sb = const.tile([P, H_LOC, HIDDEN], bf16)
            bq_sb = const.tile([P, H_LOC], fp32)
            bk_sb = const.tile([P, H_LOC], fp32)
            mask_sb = const.tile([P, B, NKT], fp32)
            ones_sb = const.tile([P, 1], bf16)
            ones_row = const.tile([1, P], bf16)   # lhsT for rcp broadcast MM

            xc0 = None
            if split_in:
                for ko in range(0, KO, 4):
                    nc.sync.dma_start(wq_sb[:, ko:ko + 4, :], wqt_v[:, ko:ko + 4, :])
                if early_x:
                    # queue the first activation chunk ahead of the remaining
                    # constants so the first projection matmuls start early
                    xc0 = xch.tile([P, KO, CHW], bf16, tag="xc", name="xc")
                    for ko in range(0, KO, 4):
                        nc.sync.dma_start(
                            xc0[:, ko:ko + 4, :], xt_v[:, ko:ko + 4, 0:CHW]
                        )
                for ko in range(0, KO, 4):
                    nc.sync.dma_start(wk_sb[:, ko:ko + 4, :], wkt_v[:, ko:ko + 4, :])
                for ko in range(0, KO, 4):
                    nc.sync.dma_start(wv_sb[:, ko:ko + 4, :], wvt_v[:, ko:ko + 4, :])
            else:
                nc.sync.dma_start(wq_sb[:], wqt_v)
                nc.sync.dma_start(wk_sb[:], wkt_v)
                nc.sync.dma_start(wv_sb[:], wvt_v)
            nc.sync.dma_start(wo_sb[:], wot_v)
            nc.sync.dma_start(bq_sb[:], bq_v)
            nc.sync.dma_start(bk_sb[:], bk_v)
            nc.sync.dma_start(mask_sb[:], mask_v)
            nc.vector.memset(ones_sb[:], 1.0)
            nc.vector.memset(ones_row[:], 1.0)

            # ---- big activation buffers ---------------------------------
            qt_sb = big.tile([P, H_LOC, T], bf16)   # Q.T  (d on partitions)
            kt_sb = big.tile([P, H_LOC, T], bf16)   # K.T
            v_sb = big.tile([P, T // P, DLOC], bf16)  # V natural (t on partitions)
            ctx_sb = big.tile([P, H_LOC, T], bf16)  # ctx.T

            def emit_body():
                if sched2:
                    # Global interleave: keep ACT-independent matmul work
                    # (projections / output projection) flowing between
                    # attention blocks so exp latency never stalls PE.
                    emit_phase1(range(0, 4))
                    att_b0 = [(0, h, qi) for h in range(H_LOC) for qi in range(NQ)]
                    for i, c in enumerate(range(4, CH)):
                        emit_phase1([c])
                        for blk in att_b0[2 * i:2 * i + 2]:
                            attention(*blk)
                    att_b1 = [(1, h, qi) for h in range(H_LOC) for qi in range(NQ)]
                    for i, blk in enumerate(att_b1):
                        attention(*blk)
                        if not no_phase3:
                            outproj(2 * i, copy_eng=0)
                            outproj(2 * i + 1, copy_eng=1)
                    if not no_phase3:
                        for tt in range(T // P // 2, T // P):
                            outproj(tt, copy_eng=tt % 2)
                    return
                emit_phase1()
                if fuse:
                    # qi-granular fusion: as soon as both heads of a q-tile
                    # are done, run its output projection + store.
                    for b in range(B):
                        for qi in range(NQ):
                            for h in range(H_LOC):
                                attention(b, h, qi)
                            if not no_phase3:
                                for j in range(4):
                                    outproj(b * 16 + qi * 4 + j, copy_eng=j % 2)
                elif interleave:
                    emit_phase2([0])
                    emit_phase3(range(0, T // P // 2))
                    emit_phase2([1])
                    emit_phase3(range(T // P // 2, T // P))
                else:
                    emit_phase2()
                    emit_phase3()

            # ---- phase 1: Q/K/V projections, streamed over token chunks --
            def emit_phase1(cs=tuple(range(CH))):
              for c in cs:
                if c == 0 and xc0 is not None:
                    xc = xc0
                else:
                    xc = xch.tile([P, KO, CHW], bf16, tag="xc", name="xc")
                    xt_eng = nc.scalar if xt_gp else nc.sync
                    if split_in:
                        for ko in range(0, KO, 4):
                            xt_eng.dma_start(
                                xc[:, ko:ko + 4, :],
                                xt_v[:, ko:ko + 4, c * CHW:(c + 1) * CHW],
                            )
                    else:
                        xt_eng.dma_start(xc[:], xt_v[:, :, c * CHW:(c + 1) * CHW])

                for h in range(H_LOC):
                    hd = slice(h * P, (h + 1) * P)
                    psq = ps_tile()
                    for ko in range(KO):
                        nc.tensor.matmul(
                            psq[:], wq_sb[:, ko, hd], xc[:, ko, :],
                            start=(ko == 0), stop=(ko == KO - 1),
                        )
                    if act_lite:
                        nc.vector.tensor_scalar_add(
                            qt_sb[:, h, c * CHW:(c + 1) * CHW], psq[:],
                            bq_sb[:, h:h + 1],
                        )
                    else:
                        nc.scalar.activation(
                            qt_sb[:, h, c * CHW:(c + 1) * CHW], psq[:],
                            Ident, bias=bq_sb[:, h:h + 1],
                        )
                    psk = ps_tile()
                    for ko in range(KO):
                        nc.tensor.matmul(
                            psk[:], wk_sb[:, ko, hd], xc[:, ko, :],
                            start=(ko == 0), stop=(ko == KO - 1),
                        )
                    if act_lite:
                        nc.vector.tensor_scalar_add(
                            kt_sb[:, h, c * CHW:(c + 1) * CHW], psk[:],
                            bk_sb[:, h:h + 1],
                        )
                    else:
                        nc.scalar.activation(
                            kt_sb[:, h, c * CHW:(c + 1) * CHW], psk[:],
                            Ident, bias=bk_sb[:, h:h + 1],
                        )

                for tt in range(CHW // P):
                    psv = ps_tile()
                    for ko in range(KO):
                        nc.tensor.matmul(
                            psv[:, :DLOC], xc[:, ko, tt * P:(tt + 1) * P],
                            wv_sb[:, ko, :],
                            start=(ko == 0), stop=(ko == KO - 1),
                        )
                    if vcopy_act:
                        nc.scalar.copy(v_sb[:, c * (CHW // P) + tt, :], psv[:, :DLOC])
                    else:
                        nc.vector.tensor_copy(
                            v_sb[:, c * (CHW // P) + tt, :], psv[:, :DLOC]
                        )

            # ---- phase 2: attention for one (batch, head, q-tile) --------
            def attention(b, h, qi):
                hd = slice(h * P, (h + 1) * P)
                qs = slice(b * S + qi * 512, b * S + (qi + 1) * 512)
                ps_ctx = ps_tile()
                ps_sum = ps_tile()
                pts = []
                accs = [None, None]
                aeng = None
                if sums_acc is not None:
                    aeng = nc.gpsimd if sums_acc == "gpsimd" else nc.vector
                def emit_st_exp(kt):
                    ks = slice(b * S + kt * P, b * S + (kt + 1) * P)
                    ps_s = ps_tile()
                    nc.tensor.matmul(
                        ps_s[:], kt_sb[:, h, ks], qt_sb[:, h, qs],
                        start=True, stop=True,
                    )
                    pt = ptp.tile([P, 512], bf16, tag="pt", name="pt")
                    nc.scalar.activation(
                        pt[:], ps_s[:], Exp,
                        bias=mask_sb[:, b, kt:kt + 1], scale=SCALE,
                    )
                    pts.append(pt)

                def emit_consume(kt):
                    pt = pts[kt]
                    nc.tensor.matmul(
                        ps_ctx[:], v_sb[:, b * NKT + kt, hd], pt[:],
                        start=(kt == 0), stop=(kt == NKT - 1),
                    )
                    if no_sums:
                        return
                    if sums_acc is not None:
                        # two interleaved accumulators trail the exps
                        i = kt % 2
                        if accs[i] is None:
                            accs[i] = ptp.tile(
                                [P, 512], bf16, tag=f"sacc{i}",
                                name=f"sacc{i}", bufs=2,
                            )
                            aeng.tensor_copy(accs[i][:], pt[:])
                        else:
                            aeng.tensor_add(accs[i][:], accs[i][:], pt[:])
                    elif not sums_defer and sums_tree is None:
                        nc.tensor.matmul(
                            ps_sum[0:1, :], ones_sb[:], pt[:],
                            start=(kt == 0), stop=(kt == NKT - 1),
                        )

                for kt in range(NKT):
                    emit_st_exp(kt)
                    if not no_attn and kt >= stagger:
                        emit_consume(kt - stagger)
                if not no_attn:
                    for kt in range(NKT - stagger, NKT):
                        emit_consume(kt)
                if no_attn:
                    return
                if sums_acc is not None and not no_sums:
                    aeng.tensor_add(accs[0][:], accs[0][:], accs[1][:])
                    nc.tensor.matmul(
                        ps_sum[0:1, :], ones_sb[:], accs[0][:],
                        start=True, stop=True,
                    )
                if sums_defer and not no_sums:
                    for kt in range(NKT):
                        nc.tensor.matmul(
                            ps_sum[0:1, :], ones_sb[:], pts[kt][:],
                            start=(kt == 0), stop=(kt == NKT - 1),
                        )
                if sums_tree is not None and not no_sums:
                    # Pairwise-add the 16 exp tiles on a non-PE engine, then a
                    # single ones-matmul does the partition reduction.
                    teng = nc.gpsimd if sums_tree == "gpsimd" else nc.vector
                    lvl = list(pts)
                    li = 0
                    while len(lvl) > 1:
                        nxt = []
                        for i in range(0, len(lvl), 2):
                            t = ptp.tile(
                                [P, 512], bf16, tag=f"tl{li}", name=f"tl{li}",
                                bufs=(10 if li == 0 else 5),
                            )
                            teng.tensor_add(t[:], lvl[i][:], lvl[i + 1][:])
                            nxt.append(t)
                        lvl = nxt
                        li += 1
                    nc.tensor.matmul(
                        ps_sum[0:1, :], ones_sb[:], lvl[0][:],
                        start=True, stop=True,
                    )
                if no_sums:
                    nc.vector.tensor_copy(ctx_sb[:, h, qs], ps_ctx[:])
                    return
                rcp = nrm.tile([1, 512], fp32, tag="rcp", name="rcp")
                nc.vector.reciprocal(rcp[:], ps_sum[0:1, 0:512])
                if norm3:
                    # Broadcast 1/sums across partitions with one K=1 matmul
                    # (ones_row.T @ rcp) — no DMA round trip on the critical
                    # path to ctx_sb.
                    rcpb = nrm.tile([1, 512], bf16, tag="rcpb", name="rcpb")
                    nc.vector.tensor_copy(rcpb[:], rcp[:])
                    ps_rbc = ps_tile()
                    nc.tensor.matmul(
                        ps_rbc[:], ones_row[:], rcpb[:], start=True, stop=True,
                    )
                    ctxu = nrm.tile([P, 512], fp32, tag="ctxu", name="ctxu")
                    nc.vector.tensor_copy(ctxu[:], ps_ctx[:])
                    nc.vector.tensor_mul(ctx_sb[:, h, qs], ctxu[:], ps_rbc[:])
                    return
                rbc = nrm.tile([P, 512], fp32, tag="rbc", name="rbc")
                rdr = dscr.tile([1, 512], fp32, tag="rdr", name="rdr")
                dma_eng = nc.scalar if norm_gp else nc.sync
                if norm2:
                    # Drain the ctx psum to SBUF right away (frees the
                    # bank); the reciprocal broadcast (DRAM bounce)
                    # happens off the critical path.
                    ctxu = nrm.tile([P, 512], fp32, tag="ctxu", name="ctxu")
                    nc.vector.tensor_copy(ctxu[:], ps_ctx[:])
                    dma_eng.dma_start(rdr[:], rcp[:])
                    dma_eng.dma_start(rbc[:], rdr[:].to_broadcast((P, 512)))
                    nc.vector.tensor_mul(ctx_sb[:, h, qs], ctxu[:], rbc[:])
                else:
                    dma_eng.dma_start(rdr[:], rcp[:])
                    dma_eng.dma_start(rbc[:], rdr[:].to_broadcast((P, 512)))
                    nc.vector.tensor_mul(ctx_sb[:, h, qs], ps_ctx[:], rbc[:])

            def emit_phase2(bs=tuple(range(B))):
                for b in bs:
                    for h in range(H_LOC):
                        for qi in range(NQ):
                            attention(b, h, qi)

            # ---- phase 3: partial output projection ----------------------
            def outproj(tt, copy_eng=0):
                ts_ = slice(tt * P, (tt + 1) * P)
                for oi in range(HIDDEN // 512):
                    os_ = slice(oi * 512, (oi + 1) * 512)
                    ps_o = ps_tile()
                    for h in range(H_LOC):
                        nc.tensor.matmul(
                            ps_o[:], ctx_sb[:, h, ts_], wo_sb[:, h, os_],
                            start=(h == 0), stop=(h == H_LOC - 1),
                        )
                    ob = obp.tile([P, 512], out_dt, tag="ob", name="ob")
                    if not act_lite and (copy_eng + oi) % 2:
                        nc.scalar.copy(ob[:], ps_o[:])
                    else:
                        nc.vector.tensor_copy(ob[:], ps_o[:])
                    if not no_out_dma:
                        nc.sync.dma_start(out_d[ts_, os_], ob[:])

            def emit_phase3(tts=tuple(range(T // P))):
                if no_phase3:
                    return
                for tt in tts:
                    outproj(tt)

            if loop_k is None:
                emit_body()
            else:
                with tc.For_i(0, loop_k, 1):
                    emit_body()

    _split_multi_waits(nc)
    return nc


def _build_nc_v2(loop_k=None, **opts):
    """v2 kernel (requires attention_mask == 0, checked host-side; nonzero
    masks fall back to the v1 build).

    Changes vs v1 (sim 425us -> 360us, HW slope 625us -> 507us/iter):
      - softmax denominators: running bf16 accumulator on DVE (2x SBUF
        mode, ~594ns per [128,1024] add) + two accumulating ones-matmuls
        per block on the accumulator halves -- removes 240 of the 256
        per-kt ones-matmuls from the PE (-51us PE, -480 instructions).
      - exps: one ACT instruction per TWO key tiles ([128,1024] across two
        PSUM banks, no bias; scale folds the 1/sqrt(d)) -- ACT per block
        9.8us -> 8.3us and half the instructions.
      - kt-granular filler interleave: batch-1 projection chunks ride
        inside batch-0 attention blocks, out-projection tiles inside
        batch-1 blocks (pair order [3,0,1,2] so every pair has ready
        fillers), so PE never waits on ACT exps.
      - normalization: reciprocal broadcast across partitions via a K=1
        ones-matmul on PE (213ns) instead of a DRAM-bounce DMA.
      - out-projection drains split ACT/DVE; stores on the SP queue
        (tail: both queues, [P,1024] double-tiles).
      - the final attention block runs as two 256-wide q sub-blocks, so
        the out-projections of its first half run as fillers of the
        second half -- halves the closing tail.
      - startup: 2-ko DMA slabs on both queues; chunk-0 matmuls consume
        slabs as they land (first matmul at ~4us).

    PSUM budget (8 banks): st [P,1024]x2 = 4 banks (score pairs) +
    per-role [P,512] rings: ctx x1, sums x1, fill x2 (chunk projections /
    out-projection tiles) -- role separation keeps the block-long ctx
    hold from blocking filler allocations.
    """
    import concourse.bass as bass
    import concourse.mybir as mybir
    import concourse.tile as tile

    stagger = opts.get("stagger", 6)
    xch_bufs = opts.get("xch_bufs", 2)
    pt_bufs = opts.get("pt_bufs", 8)
    tree_eng = opts.get("tree_eng", "vector")
    norm_mode = opts.get("norm_mode", "pe")  # dma | pe
    store_split = opts.get("store_split", True)

    fp32 = mybir.dt.float32
    bf16 = mybir.dt.bfloat16

    nc = bass.Bass()

    xt_d = nc.dram_tensor("xt", [HIDDEN, T], bf16, kind="ExternalInput")
    wqt_d = nc.dram_tensor("wqt", [HIDDEN, DLOC], bf16, kind="ExternalInput")
    wkt_d = nc.dram_tensor("wkt", [HIDDEN, DLOC], bf16, kind="ExternalInput")
    wvt_d = nc.dram_tensor("wvt", [HIDDEN, DLOC], bf16, kind="ExternalInput")
    wot_d = nc.dram_tensor("wot", [DLOC, HIDDEN], bf16, kind="ExternalInput")
    bq_d = nc.dram_tensor("bq", [DLOC], fp32, kind="ExternalInput")
    bk_d = nc.dram_tensor("bk", [DLOC], fp32, kind="ExternalInput")
    out_d = nc.dram_tensor("out", [T, HIDDEN], bf16, kind="ExternalOutput")

    xt_v = xt_d[:].rearrange("(ko p) t -> p ko t", p=P)
    wqt_v = wqt_d[:].rearrange("(ko p) d -> p ko d", p=P)
    wkt_v = wkt_d[:].rearrange("(ko p) d -> p ko d", p=P)
    wvt_v = wvt_d[:].rearrange("(ko p) d -> p ko d", p=P)
    wot_v = wot_d[:].rearrange("(h p) o -> p h o", p=P)
    bq_v = bq_d[:].rearrange("(h p) -> p h", p=P)
    bk_v = bk_d[:].rearrange("(h p) -> p h", p=P)

    with tile.TileContext(nc) as tc:
        with (
            tc.tile_pool(name="const", bufs=1) as const,
            tc.tile_pool(name="big", bufs=1) as big,
            tc.tile_pool(name="xch", bufs=xch_bufs) as xch,
            tc.tile_pool(name="ptp", bufs=pt_bufs) as ptp,
            tc.tile_pool(name="trp", bufs=2) as trp,
            tc.tile_pool(name="nrm", bufs=3) as nrm,
            tc.tile_pool(name="ob", bufs=opts.get("obp_bufs", 8)) as obp,
            tc.tile_pool(name="ps", bufs=4, space="PSUM") as psp,
            tc.tile_pool(name="st", bufs=2, space="PSUM") as stp,
            tc.tile_pool(name="dscr", bufs=4, space="DRAM") as dscr,
        ):
            Exp = mybir.ActivationFunctionType.Exp
            teng = nc.gpsimd if tree_eng == "gpsimd" else nc.vector

            noctxu = opts.get("noctxu", False)

            def ctx_tile():
                return psp.tile([P, 512], fp32, tag="ctx", name="ctx",
                                bufs=2 if noctxu else 1)

            def sum_tile():
                if noctxu:
                    return stp.tile([P, 1024], fp32, tag="st", name="st")
                return psp.tile([P, 512], fp32, tag="sum", name="sum", bufs=1)

            def fill_tile():
                return psp.tile([P, 512], fp32, tag="fill", name="fill", bufs=2)

            def st_tile():
                return stp.tile([P, 1024], fp32, tag="st", name="st")

            # ---- resident constants -------------------------------------
            wq_sb = const.tile([P, KO, DLOC], bf16)
            wk_sb = const.tile([P, KO, DLOC], bf16)
            wv_sb = const.tile([P, KO, DLOC], bf16)
            wo_sb = const.tile([P, H_LOC, HIDDEN], bf16)
            bq_sb = const.tile([P, H_LOC], fp32)
            bk_sb = const.tile([P, H_LOC], fp32)
            ones_sb = const.tile([P, 1], bf16)
            ones_row = const.tile([1, P], bf16)

            # startup: first x chunk on the ACT+DVE DMA queues, weights on SP
            xc0 = xch.tile([P, KO, CHW], bf16, tag="xc", name="xc")
            nc.scalar.dma_start(xc0[:, 0:2, :], xt_v[:, 0:2, 0:CHW])
            nc.scalar.dma_start(xc0[:, 2:4, :], xt_v[:, 2:4, 0:CHW])
            nc.scalar.dma_start(xc0[:, 4:6, :], xt_v[:, 4:6, 0:CHW])
            nc.scalar.dma_start(xc0[:, 6:8, :], xt_v[:, 6:8, 0:CHW])
            nc.sync.dma_start(wq_sb[:, 0:2, :], wqt_v[:, 0:2, :])
            nc.sync.dma_start(wq_sb[:, 2:4, :], wqt_v[:, 2:4, :])
            nc.sync.dma_start(bq_sb[:], bq_v)
            nc.sync.dma_start(wk_sb[:, 0:2, :], wkt_v[:, 0:2, :])
            nc.sync.dma_start(wk_sb[:, 2:4, :], wkt_v[:, 2:4, :])
            nc.sync.dma_start(bk_sb[:], bk_v)
            nc.sync.dma_start(wq_sb[:, 4:8, :], wqt_v[:, 4:8, :])
            nc.sync.dma_start(wk_sb[:, 4:8, :], wkt_v[:, 4:8, :])
            nc.sync.dma_start(xc0[:, 8:12, :], xt_v[:, 8:12, 0:CHW])
            nc.sync.dma_start(wq_sb[:, 8:12, :], wqt_v[:, 8:12, :])
            nc.sync.dma_start(wk_sb[:, 8:12, :], wkt_v[:, 8:12, :])
            nc.sync.dma_start(xc0[:, 12:16, :], xt_v[:, 12:16, 0:CHW])
            nc.sync.dma_start(wq_sb[:, 12:16, :], wqt_v[:, 12:16, :])
            nc.sync.dma_start(wk_sb[:, 12:16, :], wkt_v[:, 12:16, :])
            nc.sync.dma_start(wv_sb[:], wvt_v)
            nc.sync.dma_start(wo_sb[:], wot_v)
            nc.vector.memset(ones_sb[:], 1.0)
            nc.vector.memset(ones_row[:], 1.0)

            # ---- big activation buffers ---------------------------------
            qt_sb = big.tile([P, H_LOC, T], bf16)
            kt_sb = big.tile([P, H_LOC, T], bf16)
            v_sb = big.tile([P, T // P, DLOC], bf16)
            ctx_sb = big.tile([P, H_LOC, T], bf16)

            # ---- phase-1 chunk as a generator of PE filler units --------
            def chunk_units(c):
                if c == 0:
                    # chunk 0 consumes its ko groups as the startup DMAs
                    # land: all four Q/K accumulators advance one 4-ko slab
                    # at a time, so the first matmul issues after one weight
                    # group + one x group instead of the full chunk.
                    xc = xc0
                    cs = slice(0, CHW)
                    accs = {}
                    slabs = [(0, 2), (2, 4), (4, 6), (6, 8), (8, 12),
                             (12, 16)]
                    for lo, hi in slabs:
                        for h in range(H_LOC):
                            hd = slice(h * P, (h + 1) * P)
                            for w, wsb in ((0, wq_sb), (1, wk_sb)):
                                psx = accs.get((h, w))
                                if psx is None:
                                    # phase-A-only: the four chunk-0
                                    # accumulators borrow one bank from
                                    # each per-role ring
                                    mk = [fill_tile, fill_tile, ctx_tile,
                                          (fill_tile if noctxu else sum_tile)
                                          ][2 * h + w]
                                    psx = accs[(h, w)] = mk()
                                for ko in range(lo, hi):
                                    nc.tensor.matmul(
                                        psx[:], wsb[:, ko, hd], xc[:, ko, :],
                                        start=(ko == 0), stop=(ko == KO - 1),
                                    )
                                    yield
                                if hi == KO:
                                    dst, bias = ((qt_sb, bq_sb) if w == 0
                                                 else (kt_sb, bk_sb))
                                    nc.vector.tensor_scalar_add(
                                        dst[:, h, cs], psx[:], bias[:, h:h + 1]
                                    )
                else:
                    xc = xch.tile([P, KO, CHW], bf16, tag="xc", name="xc")
                    nc.sync.dma_start(xc[:], xt_v[:, :, c * CHW:(c + 1) * CHW])
                    cs = slice(c * CHW, (c + 1) * CHW)
                    for h in range(H_LOC):
                        hd = slice(h * P, (h + 1) * P)
                        psq = fill_tile()
                        for ko in range(KO):
                            nc.tensor.matmul(
                                psq[:], wq_sb[:, ko, hd], xc[:, ko, :],
                                start=(ko == 0), stop=(ko == KO - 1),
                            )
                            yield
                        nc.vector.tensor_scalar_add(
                            qt_sb[:, h, cs], psq[:], bq_sb[:, h:h + 1]
                        )
                        psk = fill_tile()
                        for ko in range(KO):
                            nc.tensor.matmul(
                                psk[:], wk_sb[:, ko, hd], xc[:, ko, :],
                                start=(ko == 0), stop=(ko == KO - 1),
                            )
                            yield
                        nc.vector.tensor_scalar_add(
                            kt_sb[:, h, cs], psk[:], bk_sb[:, h:h + 1]
                        )
                for tp in range(CHW // P // 2):  # two token tiles per psum
                    psv = fill_tile()
                    for tt in range(2):
                        for ko in range(KO):
                            nc.tensor.matmul(
                                psv[:, tt * DLOC:(tt + 1) * DLOC],
                                xc[:, ko, (2 * tp + tt) * P:(2 * tp + tt + 1) * P],
                                wv_sb[:, ko, :],
                                start=(ko == 0), stop=(ko == KO - 1),
                            )
                            if ko % 2:
                                yield
                    nc.vector.tensor_copy(
                        v_sb[:, c * (CHW // P) + 2 * tp:
                             c * (CHW // P) + 2 * tp + 2, :],
                        psv[:],
                    )

            act_ok = not opts.get("no_act", False)

            # ---- out-projection units (drain split ACT/DVE, bf16 store) -
            def outproj_units(tts, tail=False, act_drains=(0, 3, 5)):
                n = 0
                for tt in tts:
                    ts_ = slice(tt * P, (tt + 1) * P)
                    for oi in range(HIDDEN // 512):
                        os_ = slice(oi * 512, (oi + 1) * 512)
                        ps_o = fill_tile()
                        for h in range(H_LOC):
                            nc.tensor.matmul(
                                ps_o[:], ctx_sb[:, h, ts_], wo_sb[:, h, os_],
                                start=(h == 0), stop=(h == H_LOC - 1),
                            )
                        ob = obp.tile([P, 512], bf16, tag="ob", name="ob")
                        if act_ok and n % 8 in act_drains:
                            nc.scalar.copy(ob[:], ps_o[:])
                        else:
                            nc.vector.tensor_copy(ob[:], ps_o[:])
                        eng = nc.scalar if (tail and store_split and n % 2) \
                            else nc.sync
                        eng.dma_start(out_d[ts_, os_], ob[:])
                        n += 1
                        yield

            def pull(fill, n):
                for _ in range(n):
                    if next(fill, None) is None:
                        return

            # ---- attention block ----------------------------------------
            def attention(b, h, qi, fill, ppk=3, norm=None, qw=512, qoff=0):
                hd = slice(h * P, (h + 1) * P)
                q0 = b * S + qi * 512 + qoff
                qs = slice(q0, q0 + qw)
                ps_ctx = ctx_tile()
                pts = []
                acc = None

                def consume(kt):
                    nc.tensor.matmul(
                        ps_ctx[:, 0:qw], v_sb[:, b * NKT + kt, hd],
                        pts[kt // 2][:, (kt % 2) * qw:(kt % 2 + 1) * qw],
                        start=(kt == 0), stop=(kt == NKT - 1),
                    )

                for kp in range(NKT // 2):
                    ps_s = st_tile()
                    for j in range(2):
                        ks = slice(b * S + (2 * kp + j) * P,
                                   b * S + (2 * kp + j + 1) * P)
                        nc.tensor.matmul(
                            ps_s[:, j * qw:(j + 1) * qw],
                            kt_sb[:, h, ks], qt_sb[:, h, qs],
                            start=True, stop=True,
                        )
                    pt = ptp.tile([P, 1024], bf16, tag="pt", name="pt")
                    nc.scalar.activation(
                        pt[:, 0:2 * qw], ps_s[:, 0:2 * qw], Exp, scale=SCALE
                    )
                    pts.append(pt)
                    if kp == 1:
                        acc = trp.tile([P, 1024], bf16, tag="acc",
                                       name="acc", bufs=3)
                        teng.tensor_add(
                            acc[:, 0:2 * qw], pts[0][:, 0:2 * qw],
                            pts[1][:, 0:2 * qw],
                        )
                    elif kp > 1:
                        teng.tensor_add(
                            acc[:, 0:2 * qw], acc[:, 0:2 * qw],
                            pts[kp][:, 0:2 * qw],
                        )
                    for kt in (2 * kp, 2 * kp + 1):
                        if kt >= stagger:
                            consume(kt - stagger)
                    pull(fill, ppk)
                for kt in range(NKT - stagger, NKT):
                    consume(kt)
                if noctxu:
                    ctxu = ps_ctx
                else:
                    ctxu = nrm.tile([P, 512], fp32, tag="ctxu", name="ctxu")
                    nc.vector.tensor_copy(ctxu[:, 0:qw], ps_ctx[:, 0:qw])
                pull(fill, 2)
                ps_sum = sum_tile()
                nc.tensor.matmul(
                    ps_sum[0:1, 0:qw], ones_sb[:], acc[:, 0:qw],
                    start=True, stop=False,
                )
                nc.tensor.matmul(
                    ps_sum[0:1, 0:qw], ones_sb[:], acc[:, qw:2 * qw],
                    start=False, stop=True,
                )
                pull(fill, 1)
                rcp = nrm.tile([1, 512], fp32, tag="rcp", name="rcp")
                nc.vector.reciprocal(rcp[0:1, 0:qw], ps_sum[0:1, 0:qw])
                if (norm or norm_mode) == "pe":
                    rcpb = nrm.tile([1, 512], bf16, tag="rcpb", name="rcpb")
                    nc.vector.tensor_copy(rcpb[0:1, 0:qw], rcp[0:1, 0:qw])
                    ps_rbc = sum_tile()
                    nc.tensor.matmul(
                        ps_rbc[:, 0:qw], ones_row[:], rcpb[0:1, 0:qw],
                        start=True, stop=True,
                    )
                    nc.vector.tensor_mul(
                        ctx_sb[:, h, qs], ctxu[:, 0:qw], ps_rbc[:, 0:qw]
                    )
                else:
                    rbc = nrm.tile([P, 512], fp32, tag="rbc", name="rbc")
                    rdr = dscr.tile([1, 512], fp32, tag="rdr", name="rdr")
                    nc.gpsimd.dma_start(rdr[:], rcp[:])
                    nc.gpsimd.dma_start(rbc[:], rdr[:].to_broadcast((P, 512)))
                    nc.vector.tensor_mul(ctx_sb[:, h, qs], ctxu[:], rbc[:])

            def emit_body():
                # phase A: projection chunks 0-3 (batch-0 tokens)
                for c in range(4):
                    for _ in chunk_units(c):
                        pass
                # phase B: batch-0 attention; chunk 4+qi and ready
                # out-projections as filler
                for qi in range(NQ):
                    def b_fill(qi=qi):
                        yield from chunk_units(4 + qi)
                        if qi >= 1:
                            yield from outproj_units(
                                range(4 * (qi - 1), 4 * qi))
                    fill = b_fill()
                    attention(0, 0, qi, fill)
                    attention(0, 1, qi, fill)
                    for _ in fill:
                        pass
                # phase C: batch-1 attention, pairs ordered [3,0,1,2] so
                # every pair has ready out-projection fillers: pair 3 uses
                # batch-0 tiles 12-15, pair 0 then uses pair 3's tiles
                # 28-31, pair 1 uses pair 0's, pair 2 uses pair 1's.
                # Tail = pair 2's tiles 24-27.
                fills = {3: range(12, 16), 0: range(28, 32),
                         1: range(16, 20), 2: range(20, 24)}
                for i, qi in enumerate([3, 0, 1, 2]):
                    tts = list(fills[qi])
                    f1 = outproj_units(tts[:2])
                    attention(1, 0, qi, f1, ppk=1,
                              norm="pe" if i == 3 else None)
                    for _ in f1:
                        pass
                    if i < 3:
                        f2 = outproj_units(tts[2:])
                        attention(1, 1, qi, f2, ppk=1)
                        for _ in f2:
                            pass
                    else:
                        # final block split into two 256-wide q sub-blocks:
                        # sub1 finalizes ctx for tiles 24-25, which then
                        # run as sub2's fillers -- halves the closing tail
                        f2 = outproj_units(tts[2:])
                        attention(1, 1, qi, f2, ppk=1, norm="pe",
                                  qw=256, qoff=0)
                        for _ in f2:
                            pass
                        f3 = outproj_units(range(24, 26))
                        attention(1, 1, qi, f3, ppk=1, norm="pe",
                                  qw=256, qoff=256)
                        for _ in f3:
                            pass
                # tail: two-oi [P,1024] units on the now-idle st ring,
                # drains alternating ACT/DVE, stores on both queues
                n = 0
                for tt in range(26, 28):
                    ts_ = slice(tt * P, (tt + 1) * P)
                    for oj in range(2):
                        os2 = slice(oj * 1024, (oj + 1) * 1024)
                        ps_o = st_tile()
                        for half in range(2):
                            hs = slice(half * 512, (half + 1) * 512)
                            ow = slice(oj * 1024 + half * 512,
                                       oj * 1024 + (half + 1) * 512)
                            for h in range(H_LOC):
                                nc.tensor.matmul(
                                    ps_o[:, hs], ctx_sb[:, h, ts_],
                                    wo_sb[:, h, ow],
                                    start=(h == 0), stop=(h == H_LOC - 1),
                                )
                        ob = obp.tile([P, 1024], bf16, tag="ob2", name="ob2",
                                      bufs=4)
                        if (n % 2) and act_ok:
                            nc.scalar.copy(ob[:], ps_o[:])
                        else:
                            nc.vector.tensor_copy(ob[:], ps_o[:])
                        eng = nc.scalar if n % 2 else nc.sync
                        eng.dma_start(out_d[ts_, os2], ob[:])
                        n += 1

            if loop_k is None:
                emit_body()
            else:
                with tc.For_i(0, loop_k, 1):
                    emit_body()

    _split_multi_waits(nc)
    return nc


def _build_nc_v2(loop_k=None, **opts):
    """v2 kernel (requires attention_mask == 0, checked host-side; nonzero
    masks fall back to the v1 build).

    Changes vs v1 (sim 425us -> 360us, HW slope 625us -> 507us/iter):
      - softmax denominators: running bf16 accumulator on DVE (2x SBUF
        mode, ~594ns per [128,1024] add) + two accumulating ones-matmuls
        per block on the accumulator halves -- removes 240 of the 256
        per-kt ones-matmuls from the PE (-51us PE, -480 instructions).
      - exps: one ACT instruction per TWO key tiles ([128,1024] across two
        PSUM banks, no bias; scale folds the 1/sqrt(d)) -- ACT per block
        9.8us -> 8.3us and half the instructions.
      - kt-granular filler interleave: batch-1 projection chunks ride
        inside batch-0 attention blocks, out-projection tiles inside
        batch-1 blocks (pair order [3,0,1,2] so every pair has ready
        fillers), so PE never waits on ACT exps.
      - normalization: reciprocal broadcast across partitions via a K=1
        ones-matmul on PE (213ns) instead of a DRAM-bounce DMA.
      - out-projection drains split ACT/DVE; stores on the SP queue
        (tail: both queues, [P,1024] double-tiles).
      - the final attention block runs as two 256-wide q sub-blocks, so
        the out-projections of its first half run as fillers of the
        second half -- halves the closing tail.
      - startup: 2-ko DMA slabs on both queues; chunk-0 matmuls consume
        slabs as they land (first matmul at ~4us).

    PSUM budget (8 banks): st [P,1024]x2 = 4 banks (score pairs) +
    per-role [P,512] rings: ctx x1, sums x1, fill x2 (chunk projections /
    out-projection tiles) -- role separation keeps the block-long ctx
    hold from blocking filler allocations.
    """
    import concourse.bass as bass
    import concourse.mybir as mybir
    import concourse.tile as tile

    stagger = opts.get("stagger", 6)
    xch_bufs = opts.get("xch_bufs", 2)
    pt_bufs = opts.get("pt_bufs", 8)
    tree_eng = opts.get("tree_eng", "vector")
    norm_mode = opts.get("norm_mode", "pe")  # dma | pe
    store_split = opts.get("store_split", True)

    fp32 = mybir.dt.float32
    bf16 = mybir.dt.bfloat16

    nc = bass.Bass()

    xt_d = nc.dram_tensor("xt", [HIDDEN, T], bf16, kind="ExternalInput")
    wqt_d = nc.dram_tensor("wqt", [HIDDEN, DLOC], bf16, kind="ExternalInput")
    wkt_d = nc.dram_tensor("wkt", [HIDDEN, DLOC], bf16, kind="ExternalInput")
    wvt_d = nc.dram_tensor("wvt", [HIDDEN, DLOC], bf16, kind="ExternalInput")
    wot_d = nc.dram_tensor("wot", [DLOC, HIDDEN], bf16, kind="ExternalInput")
    bq_d = nc.dram_tensor("bq", [DLOC], fp32, kind="ExternalInput")
    bk_d = nc.dram_tensor("bk", [DLOC], fp32, kind="ExternalInput")
    out_d = nc.dram_tensor("out", [T, HIDDEN], bf16, kind="ExternalOutput")

    xt_v = xt_d[:].rearrange("(ko p) t -> p ko t", p=P)
    wqt_v = wqt_d[:].rearrange("(ko p) d -> p ko d", p=P)
    wkt_v = wkt_d[:].rearrange("(ko p) d -> p ko d", p=P)
    wvt_v = wvt_d[:].rearrange("(ko p) d -> p ko d", p=P)
    wot_v = wot_d[:].rearrange("(h p) o -> p h o", p=P)
    bq_v = bq_d[:].rearrange("(h p) -> p h", p=P)
    bk_v = bk_d[:].rearrange("(h p) -> p h", p=P)

    with tile.TileContext(nc) as tc:
        with (
            tc.tile_pool(name="const", bufs=1) as const,
            tc.tile_pool(name="big", bufs=1) as big,
            tc.tile_pool(name="xch", bufs=xch_bufs) as xch,
            tc.tile_pool(name="ptp", bufs=pt_bufs) as ptp,
            tc.tile_pool(name="trp", bufs=2) as trp,
            tc.tile_pool(name="nrm", bufs=3) as nrm,
            tc.tile_pool(name="ob", bufs=opts.get("obp_bufs", 8)) as obp,
            tc.tile_pool(name="ps", bufs=4, space="PSUM") as psp,
            tc.tile_pool(name="st", bufs=2, space="PSUM") as stp,
            tc.tile_pool(name="dscr", bufs=4, space="DRAM") as dscr,
        ):
            Exp = mybir.ActivationFunctionType.Exp
            teng = nc.gpsimd if tree_eng == "gpsimd" else nc.vector

            noctxu = opts.get("noctxu", False)

            def ctx_tile():
                return psp.tile([P, 512], fp32, tag="ctx", name="ctx",
                                bufs=2 if noctxu else 1)

            def sum_tile():
                if noctxu:
                    return stp.tile([P, 1024], fp32, tag="st", name="st")
                return psp.tile([P, 512], fp32, tag="sum", name="sum", bufs=1)

            def fill_tile():
                return psp.tile([P, 512], fp32, tag="fill", name="fill", bufs=2)

            def st_tile():
                return stp.tile([P, 1024], fp32, tag="st", name="st")

            # ---- resident constants -------------------------------------
            wq_sb = const.tile([P, KO, DLOC], bf16)
            wk_sb = const.tile([P, KO, DLOC], bf16)
            wv_sb = const.tile([P, KO, DLOC], bf16)
            wo_sb = const.tile([P, H_LOC, HIDDEN], bf16)
            bq_sb = const.tile([P, H_LOC], fp32)
            bk_sb = const.tile([P, H_LOC], fp32)
            ones_sb = const.tile([P, 1], bf16)
            ones_row = const.tile([1, P], bf16)

            # startup: first x chunk on the ACT+DVE DMA queues, weights on SP
            xc0 = xch.tile([P, KO, CHW], bf16, tag="xc", name="xc")
            nc.scalar.dma_start(xc0[:, 0:2, :], xt_v[:, 0:2, 0:CHW])
            nc.scalar.dma_start(xc0[:, 2:4, :], xt_v[:, 2:4, 0:CHW])
            nc.scalar.dma_start(xc0[:, 4:6, :], xt_v[:, 4:6, 0:CHW])
            nc.scalar.dma_start(xc0[:, 6:8, :], xt_v[:, 6:8, 0:CHW])
            nc.sync.dma_start(wq_sb[:, 0:2, :], wqt_v[:, 0:2, :])
            nc.sync.dma_start(wq_sb[:, 2:4, :], wqt_v[:, 2:4, :])
            nc.sync.dma_start(bq_sb[:], bq_v)
            nc.sync.dma_start(wk_sb[:, 0:2, :], wkt_v[:, 0:2, :])
            nc.sync.dma_start(wk_sb[:, 2:4, :], wkt_v[:, 2:4, :])
            nc.sync.dma_start(bk_sb[:], bk_v)
            nc.sync.dma_start(wq_sb[:, 4:8, :], wqt_v[:, 4:8, :])
            nc.sync.dma_start(wk_sb[:, 4:8, :], wkt_v[:, 4:8, :])
            nc.sync.dma_start(xc0[:, 8:12, :], xt_v[:, 8:12, 0:CHW])
            nc.sync.dma_start(wq_sb[:, 8:12, :], wqt_v[:, 8:12, :])
            nc.sync.dma_start(wk_sb[:, 8:12, :], wkt_v[:, 8:12, :])
            nc.sync.dma_start(xc0[:, 12:16, :], xt_v[:, 12:16, 0:CHW])
            nc.sync.dma_start(wq_sb[:, 12:16, :], wqt_v[:, 12:16, :])
            nc.sync.dma_start(wk_sb[:, 12:16, :], wkt_v[:, 12:16, :])
            nc.sync.dma_start(wv_sb[:], wvt_v)
            nc.sync.dma_start(wo_sb[:], wot_v)
            nc.vector.memset(ones_sb[:], 1.0)
            nc.vector.memset(ones_row[:], 1.0)

            # ---- big activation buffers ---------------------------------
            qt_sb = big.tile([P, H_LOC, T], bf16)
            kt_sb = big.tile([P, H_LOC, T], bf16)
            v_sb = big.tile([P, T // P, DLOC], bf16)
            ctx_sb = big.tile([P, H_LOC, T], bf16)

            # ---- phase-1 chunk as a generator of PE filler units --------
            def chunk_units(c):
                if c == 0:
                    # chunk 0 consumes its ko groups as the startup DMAs
                    # land: all four Q/K accumulators advance one 4-ko slab
                    # at a time, so the first matmul issues after one weight
                    # group + one x group instead of the full chunk.
                    xc = xc0
                    cs = slice(0, CHW)
                    accs = {}
                    slabs = [(0, 2), (2, 4), (4, 6), (6, 8), (8, 12),
                             (12, 16)]
                    for lo, hi in slabs:
                        for h in range(H_LOC):
                            hd = slice(h * P, (h + 1) * P)
                            for w, wsb in ((0, wq_sb), (1, wk_sb)):
                                psx = accs.get((h, w))
                                if psx is None:
                                    # phase-A-only: the four chunk-0
                                    # accumulators borrow one bank from
                                    # each per-role ring
                                    mk = [fill_tile, fill_tile, ctx_tile,
                                          (fill_tile if noctxu else sum_tile)
                                          ][2 * h + w]
                                    psx = accs[(h, w)] = mk()
                                for ko in range(lo, hi):
                                    nc.tensor.matmul(
                                        psx[:], wsb[:, ko, hd], xc[:, ko, :],
                                        start=(ko == 0), stop=(ko == KO - 1),
                                    )
                                    yield
                                if hi == KO:
                                    dst, bias = ((qt_sb, bq_sb) if w == 0
                                                 else (kt_sb, bk_sb))
                                    nc.vector.tensor_scalar_add(
                                        dst[:, h, cs], psx[:], bias[:, h:h + 1]
                                    )
                else:
                    xc = xch.tile([P, KO, CHW], bf16, tag="xc", name="xc")
                    nc.sync.dma_start(xc[:], xt_v[:, :, c * CHW:(c + 1) * CHW])
                    cs = slice(c * CHW, (c + 1) * CHW)
                    for h in range(H_LOC):
                        hd = slice(h * P, (h + 1) * P)
                        psq = fill_tile()
                        for ko in range(KO):
                            nc.tensor.matmul(
                                psq[:], wq_sb[:, ko, hd], xc[:, ko, :],
                                start=(ko == 0), stop=(ko == KO - 1),
                            )
                            yield
                        nc.vector.tensor_scalar_add(
                            qt_sb[:, h, cs], psq[:], bq_sb[:, h:h + 1]
                        )
                        psk = fill_tile()
                        for ko in range(KO):
                            nc.tensor.matmul(
                                psk[:], wk_sb[:, ko, hd], xc[:, ko, :],
                                start=(ko == 0), stop=(ko == KO - 1),
                            )
                            yield
                        nc.vector.tensor_scalar_add(
                            kt_sb[:, h, cs], psk[:], bk_sb[:, h:h + 1]
                        )
                for tp in range(CHW // P // 2):  # two token tiles per psum
                    psv = fill_tile()
                    for tt in range(2):
                        for ko in range(KO):
                            nc.tensor.matmul(
                                psv[:, tt * DLOC:(tt + 1) * DLOC],
                                xc[:, ko, (2 * tp + tt) * P:(2 * tp + tt + 1) * P],
                                wv_sb[:, ko, :],
                                start=(ko == 0), stop=(ko == KO - 1),
                            )
                            if ko % 2:
                                yield
                    nc.vector.tensor_copy(
                        v_sb[:, c * (CHW // P) + 2 * tp:
                             c * (CHW // P) + 2 * tp + 2, :],
                        psv[:],
                    )

            act_ok = not opts.get("no_act", False)

            # ---- out-projection units (drain split ACT/DVE, bf16 store) -
            def outproj_units(tts, tail=False, act_drains=(0, 3, 5)):
                n = 0
                for tt in tts:
                    ts_ = slice(tt * P, (tt + 1) * P)
                    for oi in range(HIDDEN // 512):
                        os_ = slice(oi * 512, (oi + 1) * 512)
                        ps_o = fill_tile()
                        for h in range(H_LOC):
                            nc.tensor.matmul(
                                ps_o[:], ctx_sb[:, h, ts_], wo_sb[:, h, os_],
                                start=(h == 0), stop=(h == H_LOC - 1),
                            )
                        ob = obp.tile([P, 512], bf16, tag="ob", name="ob")
                        if act_ok and n % 8 in act_drains:
                            nc.scalar.copy(ob[:], ps_o[:])
                        else:
                            nc.vector.tensor_copy(ob[:], ps_o[:])
                        eng = nc.scalar if (tail and store_split and n % 2) \
                            else nc.sync
                        eng.dma_start(out_d[ts_, os_], ob[:])
                        n += 1
                        yield

            def pull(fill, n):
                for _ in range(n):
                    if next(fill, None) is None:
                        return

            # ---- attention block ----------------------------------------
            def attention(b, h, qi, fill, ppk=3, norm=None, qw=512, qoff=0):
                hd = slice(h * P, (h + 1) * P)
                q0 = b * S + qi * 512 + qoff
                qs = slice(q0, q0 + qw)
                ps_ctx = ctx_tile()
                pts = []
                acc = None

                def consume(kt):
                    nc.tensor.matmul(
                        ps_ctx[:, 0:qw], v_sb[:, b * NKT + kt, hd],
                        pts[kt // 2][:, (kt % 2) * qw:(kt % 2 + 1) * qw],
                        start=(kt == 0), stop=(kt == NKT - 1),
                    )

                for kp in range(NKT // 2):
                    ps_s = st_tile()
                    for j in range(2):
                        ks = slice(b * S + (2 * kp + j) * P,
                                   b * S + (2 * kp + j + 1) * P)
                        nc.tensor.matmul(
                            ps_s[:, j * qw:(j + 1) * qw],
                            kt_sb[:, h, ks], qt_sb[:, h, qs],
                            start=True, stop=True,
                        )
                    pt = ptp.tile([P, 1024], bf16, tag="pt", name="pt")
                    nc.scalar.activation(
                        pt[:, 0:2 * qw], ps_s[:, 0:2 * qw], Exp, scale=SCALE
                    )
                    pts.append(pt)
                    if kp == 1:
                        acc = trp.tile([P, 1024], bf16, tag="acc",
                                       name="acc", bufs=3)
                        teng.tensor_add(
                            acc[:, 0:2 * qw], pts[0][:, 0:2 * qw],
                            pts[1][:, 0:2 * qw],
                        )
                    elif kp > 1:
                        teng.tensor_add(
                            acc[:, 0:2 * qw], acc[:, 0:2 * qw],
                            pts[kp][:, 0:2 * qw],
                        )
                    for kt in (2 * kp, 2 * kp + 1):
                        if kt >= stagger:
                            consume(kt - stagger)
                    pull(fill, ppk)
                for kt in range(NKT - stagger, NKT):
                    consume(kt)
                if noctxu:
                    ctxu = ps_ctx
                else:
                    ctxu = nrm.tile([P, 512], fp32, tag="ctxu", name="ctxu")
                    nc.vector.tensor_copy(ctxu[:, 0:qw], ps_ctx[:, 0:qw])
                pull(fill, 2)
                ps_sum = sum_tile()
                nc.tensor.matmul(
                    ps_sum[0:1, 0:qw], ones_sb[:], acc[:, 0:qw],
                    start=True, stop=False,
                )
                nc.tensor.matmul(
                    ps_sum[0:1, 0:qw], ones_sb[:], acc[:, qw:2 * qw],
                    start=False, stop=True,
                )
                pull(fill, 1)
                rcp = nrm.tile([1, 512], fp32, tag="rcp", name="rcp")
                nc.vector.reciprocal(rcp[0:1, 0:qw], ps_sum[0:1, 0:qw])
                if (norm or norm_mode) == "pe":
                    rcpb = nrm.tile([1, 512], bf16, tag="rcpb", name="rcpb")
                    nc.vector.tensor_copy(rcpb[0:1, 0:qw], rcp[0:1, 0:qw])
                    ps_rbc = sum_tile()
                    nc.tensor.matmul(
                        ps_rbc[:, 0:qw], ones_row[:], rcpb[0:1, 0:qw],
                        start=True, stop=True,
                    )
                    nc.vector.tensor_mul(
                        ctx_sb[:, h, qs], ctxu[:, 0:qw], ps_rbc[:, 0:qw]
                    )
                else:
                    rbc = nrm.tile([P, 512], fp32, tag="rbc", name="rbc")
                    rdr = dscr.tile([1, 512], fp32, tag="rdr", name="rdr")
                    nc.gpsimd.dma_start(rdr[:], rcp[:])
                    nc.gpsimd.dma_start(rbc[:], rdr[:].to_broadcast((P, 512)))
                    nc.vector.tensor_mul(ctx_sb[:, h, qs], ctxu[:], rbc[:])

            def emit_body():
                # phase A: projection chunks 0-3 (batch-0 tokens)
                for c in range(4):
                    for _ in chunk_units(c):
                        pass
                # phase B: batch-0 attention; chunk 4+qi and ready
                # out-projections as filler
                for qi in range(NQ):
                    def b_fill(qi=qi):
                        yield from chunk_units(4 + qi)
                        if qi >= 1:
                            yield from outproj_units(
                                range(4 * (qi - 1), 4 * qi))
                    fill = b_fill()
                    attention(0, 0, qi, fill)
                    attention(0, 1, qi, fill)
                    for _ in fill:
                        pass
                # phase C: batch-1 attention, pairs ordered [3,0,1,2] so
                # every pair has ready out-projection fillers: pair 3 uses
                # batch-0 tiles 12-15, pair 0 then uses pair 3's tiles
                # 28-31, pair 1 uses pair 0's, pair 2 uses pair 1's.
                # Tail = pair 2's tiles 24-27.
                fills = {3: range(12, 16), 0: range(28, 32),
                         1: range(16, 20), 2: range(20, 24)}
                for i, qi in enumerate([3, 0, 1, 2]):
                    tts = list(fills[qi])
                    f1 = outproj_units(tts[:2])
                    attention(1, 0, qi, f1, ppk=1,
                              norm="pe" if i == 3 else None)
                    for _ in f1:
                        pass
                    if i < 3:
                        f2 = outproj_units(tts[2:])
                        attention(1, 1, qi, f2, ppk=1)
                        for _ in f2:
                            pass
                    else:
                        # final block split into two 256-wide q sub-blocks:
                        # sub1 finalizes ctx for tiles 24-25, which then
                        # run as sub2's fillers -- halves the closing tail
                        f2 = outproj_units(tts[2:])
                        attention(1, 1, qi, f2, ppk=1, norm="pe",
                                  qw=256, qoff=0)
                        for _ in f2:
                            pass
                        f3 = outproj_units(range(24, 26))
                        attention(1, 1, qi, f3, ppk=1, norm="pe",
                                  qw=256, qoff=256)
                        for _ in f3:
                            pass
                # tail: two-oi [P,1024] units on the now-idle st ring,
                # drains alternating ACT/DVE, stores on both queues
                n = 0
                for tt in range(26, 28):
                    ts_ = slice(tt * P, (tt + 1) * P)
                    last_tt = tt == 27
                    for oj in range(2):
                        os2 = slice(oj * 1024, (oj + 1) * 1024)
                        ps_o = st_tile()
                        for half in range(2):
                            hs = slice(half * 512, (half + 1) * 512)
                            ow = slice(oj * 1024 + half * 512,
                                       oj * 1024 + (half + 1) * 512)
                            for h in range(H_LOC):
                                nc.tensor.matmul(
                                    ps_o[:, hs], ctx_sb[:, h, ts_],
                                    wo_sb[:, h, ow],
                                    start=(h == 0), stop=(h == H_LOC - 1),
                                )
                        if last_tt:
                            # final tile: halve each drain/store and run
                            # them on both engines + both queues so the
                            # closing latency chain is as short as possible
                            for half in range(2):
                                hs = slice(half * 512, (half + 1) * 512)
                                ow = slice(oj * 1024 + half * 512,
                                           oj * 1024 + (half + 1) * 512)
                                ob = obp.tile([P, 512], bf16, tag="ob",
                                              name="ob")
                                if half and act_ok:
                                    nc.scalar.copy(ob[:], ps_o[:, hs])
                                    nc.scalar.dma_start(out_d[ts_, ow], ob[:])
                                else:
                                    nc.vector.tensor_copy(ob[:], ps_o[:, hs])
                                    eng = nc.scalar if half else nc.sync
                                    eng.dma_start(out_d[ts_, ow], ob[:])
                            n += 1
                            continue
                        ob = obp.tile([P, 1024], bf16, tag="ob2", name="ob2",
                                      bufs=4)
                        if n % 2 and act_ok:
                            nc.scalar.copy(ob[:], ps_o[:])
                        else:
                            nc.vector.tensor_copy(ob[:], ps_o[:])
                        eng = nc.scalar if n % 2 else nc.sync
                        eng.dma_start(out_d[ts_, os2], ob[:])
                        n += 1

            if loop_k is None:
                emit_body()
            else:
                with tc.For_i(0, loop_k, 1):
                    emit_body()

    _split_multi_waits(nc)
    return nc


# Final tuned configuration (HW-validated via slope benchmarking + cost model):
#  - out_bf16: bf16 partial outputs (halves store traffic; host sums in fp64)
#  - obp_bufs=8: deep store pipeline (phase-3 tail was store-bound)
#  - split_in: split startup DMAs so the first matmuls start early
#  - norm2: drain ctx psum early; reciprocal broadcast off the critical path
#  - act_lite: ACT engine reserved for exps; bias-adds/drains on DVE
#  - sched2 + stagger: interleave projection/attention/out-projection emission
#    and software-pipeline the attention loop so exp latency never stalls PE
_DEFAULT_OPTS = dict(
    out_bf16=True, obp_bufs=8, split_in=True, norm2=True,
    act_lite=True, sched2=True, stagger=2, early_x=True,
)


def _get_nc():
    if "nc" not in _CACHE:
        _CACHE["nc"] = _build_nc(**_DEFAULT_OPTS)
    return _CACHE["nc"]


def _get_nc_v2():
    if "nc_v2" not in _CACHE:
        _CACHE["nc_v2"] = _build_nc_v2()
    return _CACHE["nc_v2"]


def kernel(**inputs):
    hs = np.asarray(inputs["hidden_states"], dtype=np.float32)
    mask = np.asarray(inputs["attention_mask"], dtype=np.float32)
    Wq = np.asarray(inputs["Wq"], dtype=np.float32)
    bq = np.asarray(inputs["bq"], dtype=np.float32)
    Wk = np.asarray(inputs["Wk"], dtype=np.float32)
    bk = np.asarray(inputs["bk"], dtype=np.float32)
    Wv = np.asarray(inputs["Wv"], dtype=np.float32)
    bv = np.asarray(inputs["bv"], dtype=np.float32)
    Wo = np.asarray(inputs["Wo"], dtype=np.float32)
    bo = np.asarray(inputs["bo"], dtype=np.float32)

    x = hs.reshape(T, HIDDEN)
    xt = np.ascontiguousarray(x.T).astype(BF16NP)
    mask2 = np.ascontiguousarray(mask.reshape(B, S))

    # v2 kernel folds the (all-zero) attention mask away; a nonzero mask
    # takes the general v1 path with the mask as a per-key exp bias.
    use_v2 = not np.any(mask)

    in_maps = []
    for c in range(N_CORES):
        rs = slice(c * DLOC, (c + 1) * DLOC)
        m = {
            "xt": xt,
            "wqt": np.ascontiguousarray(Wq[rs, :].T).astype(BF16NP),
            "wkt": np.ascontiguousarray(Wk[rs, :].T).astype(BF16NP),
            "wvt": np.ascontiguousarray(Wv[rs, :].T).astype(BF16NP),
            "wot": np.ascontiguousarray(Wo[:, rs].T).astype(BF16NP),
            "bq": np.ascontiguousarray(bq[rs]),
            "bk": np.ascontiguousarray(bk[rs]),
        }
        if not use_v2:
            m["mask"] = mask2
        in_maps.append(m)

    from concourse.bass_utils import run_bass_kernel_spmd

    nc = _get_nc_v2() if use_v2 else _get_nc()
    trace = bool(int(os.environ.get("MHA_KERNEL_TRACE", "0")))
    res = run_bass_kernel_spmd(
        nc, in_maps, core_ids=list(range(N_CORES)), trace=trace,
        **({"trace_cores": list(range(N_CORES))} if trace else {}),
    )
    _CACHE["last_results"] = res

    out = np.sum(
        np.stack([r["out"] for r in res.results]), axis=0, dtype=np.float64
    )
    out += bv.astype(np.float64) @ Wo.T.astype(np.float64) + bo
    return out.astype(np.float32).reshape(B, S, HIDDEN)

